# revision 38
# baseline (speedup 1.0000x reference)
"""Trainium2 Bass kernel for an MQA attention block (8 q-heads, shared K/V).

Sharding: 8 cores; core c -> batch b=c//4, query rows s0=(c%4)*512 .. +512,
all 8 heads.  K/V (full sequence, per batch) are computed redundantly on each
core; no cross-core communication.

Layout strategy ("row" architecture):
 - K/V projection computed jointly in ROW layout (keys on partitions): per
   key-chunk, lhsT = xT chunk (stationary), rhs = [Wk|Wv] (moving, N=320).
   LayerNorm stats are per-partition; affines are folded into rope tables
   (k) and into Wo/bo (v).  v rows written bf16, consumed directly by the
   attention y-matmul; k rows rope'd then PE-transposed to kT.
 - Q projection in ROW layout per half-head-group (N=512), LN+rope in rows
   (affine + DQ^-0.5 folded into tables), PE-transposed to qT.
 - Attention: logits = kT.T @ qT (bf16); softcap chain is one DVE
   scalar_tensor_tensor (x0.2 + bias, writes bf16) + ACT tanh + ACT exp;
   y accumulates p.T @ [v|1|pad] (bf16, N=200) with the ones-column
   giving the softmax denominator.  The LN applies run on the scalar
   engine as Identity(x*rsig - mu*rsig); sums come free from the
   eviction's accum_out, sumsq from Square's accum_out.
   kv and q projection emission is interleaved so the Tile scheduler
   overlaps them; PSUM stays at 2+2+2 banks in that scope (a 3+3+2
   config produced wrong results - do not raise it).
 - Output projection: yatt rows (bf16) -> PE transpose (bf16) -> yT,
   matmul with bf16 Wo (v-affine + rms2 folded), add bias, DMA out.
"""

import os
import sys

for _p in ("/opt/trn_rl_repo",):
    if _p not in sys.path and os.path.isdir(_p):
        sys.path.insert(0, _p)

import numpy as np
from contextlib import ExitStack

import concourse.bass as bass
import concourse.mybir as mybir
import concourse.tile as tile
from concourse import bacc
from concourse import bass_utils

F32 = mybir.dt.float32
F32R = mybir.dt.float32r
BF16 = mybir.dt.bfloat16
NPBF16 = mybir.dt.np(mybir.dt.bfloat16)

B, S, D = 2, 2048, 1536
H, DQ, DK, DV = 8, 128, 128, 192
P = 128
SQ = S // 4          # 512 query rows per core
DC = D // P          # 12 contraction chunks
JC = S // P          # 16 key chunks
SC = SQ // P         # 4 query-row chunks
NCORES = 8
EPS_RMS = 1e-6
EPS_LN = 1e-5
SOFTCAP = 5.0
ROPE_BASE = 8192.0
HALF = DQ // 2
KV = DK + DV         # 320: joint k|v row width
VP = 200             # v row: 192 v + ones col @192 + 7 pad
HG = 4               # heads per q-projection group

# consts packing (col offsets in the [128, CW] const tensor)
C_ID = 0             # ident f32 [128]
C_BKV = 128          # kv eviction bias rep [320]
C_BQ = 448           # q eviction bias rep [1024]
C_INV = 1472         # [1/DK, 1/DV] rep
C_EPS = 1474         # EPS_LN
CW = 1476


def _r(ap):
    return ap.bitcast(F32R)


def build_program(has_kb=False, has_qb=False):
    nc = bacc.Bacc(
        "TRN2", target_bir_lowering=False, debug=False, num_devices=NCORES
    )

    def din(name, shape, dt=F32):
        return nc.dram_tensor(name, list(shape), dt, kind="ExternalInput").ap()

    xT = din("xT", (D, S), BF16)              # per-core batch
    xq_in = din("xq", (D, SQ), BF16)          # per-core q column slice
    biasT = din("biasT", (S, SQ), BF16)       # per-core, x0.2 prescaled
    ropeq = din("ropeq", (SQ, (3 if has_qb else 2) * 4 * P), BF16)  # per-core
    ropek = din("ropek", (S, (3 if has_kb else 2) * P), BF16)
    wkv = din("wkv", (D, KV), BF16)
    wq = din("wq", (D, H * DQ), BF16)
    wo = din("wo", (H * DV, D), BF16)
    consts = din("consts", (P, CW))
    bout = din("bout", (P, D))
    out = nc.dram_tensor("out", [SQ, D], F32, kind="ExternalOutput").ap()

    TT = mybir.AluOpType
    AF = mybir.ActivationFunctionType
    AX = mybir.AxisListType

    qjc0 = None  # chunks covered by this core's q slice: set via partition id?
    # Each core's q slice differs, but the PROGRAM is shared across cores.
    # xq is just a slice of xT columns; we DMA those 4 chunks twice (once into
    # the stream tile for kv, once into xq).  Cheap (3.1MB extra DMA).

    with tile.TileContext(nc) as tc, ExitStack() as ctx:
        const = ctx.enter_context(tc.tile_pool(name="const", bufs=1))
        persist = ctx.enter_context(tc.tile_pool(name="persist", bufs=1))

        cst = const.tile([P, CW], F32)
        ident = cst[:, C_ID : C_ID + P]
        ident_bf = const.tile([P, P], BF16)
        eps_ap = cst[:, C_EPS : C_EPS + 1]

        kT_sb = persist.tile([P, JC, P], BF16)         # [dk, jc, key]
        biasT_sb = persist.tile([P, JC, SQ], BF16)     # prefetched in kv
        vrow_sb = persist.tile([P, JC, VP], BF16)     # [key, jc, v|1|pad]
        qT_sb = persist.tile([P, H, SQ], BF16)         # [dq, h, q]

        nc.vector.memset(vrow_sb[:, :, DV : DV + 1], 1.0)
        nc.vector.memset(vrow_sb[:, :, DV + 1 :], 0.0)

        qres_cm = tc.tile_pool(name="qres", bufs=1)
        qres = qres_cm.__enter__()
        xq_sb = qres.tile([P, DC, SQ], BF16)

        # =====================================================
        # Phase KV
        # =====================================================
        with (
            tc.tile_pool(name="kvs", bufs=1) as kvs,
            tc.tile_pool(name="kvw", bufs=2) as kvw,
            tc.tile_pool(name="xsp", bufs=3) as xsp,
            tc.tile_pool(name="kn4p", bufs=2) as kn4p,
            tc.tile_pool(name="qw", bufs=3) as qw,
            tc.tile_pool(name="kvps", bufs=2, space="PSUM") as kvps,
            tc.tile_pool(name="qps", bufs=2, space="PSUM") as qps,
            tc.tile_pool(name="kvtp", bufs=2, space="PSUM") as kvtp,
        ):
            x3 = xT.rearrange("(c p) s -> p c s", p=P)
            xslab0 = xsp.tile([P, DC, 2 * P], BF16, tag="xslab")
            nc.sync.dma_start(xslab0[:, :, 0:P], x3[:, :, 0:P])
            nc.sync.dma_start(xslab0[:, :, P : 2 * P], x3[:, :, P : 2 * P])
            wkv_sb = kvs.tile([P, DC, KV], BF16)
            nc.sync.dma_start(
                wkv_sb[:], wkv.rearrange("(c p) f -> p c f", p=P)
            )
            nc.sync.dma_start(cst[:], consts)
            nc.vector.tensor_copy(ident_bf[:], ident)
            ropek_sb = kvs.tile([P, JC, (3 if has_kb else 2) * P], BF16)
            nc.sync.dma_start(
                ropek_sb[:], ropek.rearrange("(j p) f -> p j f", p=P)
            )
            wq_sb = qres.tile([P, DC, H * DQ], BF16)
            ropeq_sb = qres.tile([P, SC, (3 if has_qb else 2) * 4 * P], BF16)

            invn = cst[:, C_INV : C_INV + 2]

            def emit_q(g, ic):
                f0 = g * HG * DQ
                q_ps = qps.tile([P, HG * DQ], F32, tag="q_ps")
                for dc in range(DC):
                    nc.tensor.matmul(
                        q_ps[:],
                        xq_sb[:, dc, ic * P : (ic + 1) * P],
                        wq_sb[:, dc, f0 : f0 + HG * DQ],
                        start=(dc == 0),
                        stop=(dc == DC - 1),
                    )
                qr = qw.tile([P, HG * DQ], F32, tag="qr")
                nc.vector.tensor_tensor(
                    qr[:], q_ps[:],
                    cst[:, C_BQ + f0 : C_BQ + f0 + HG * DQ], TT.add
                )
                sqq = kvw.tile([P, HG * DQ], F32, tag="sqq")
                stq = qw.tile([P, 28], F32, tag="stq")
                qr3 = qr[:].rearrange("p (h f) -> p h f", h=HG)
                nc.vector.tensor_reduce(stq[:, 0:4], qr3, AX.X, TT.add)
                for t in range(HG):
                    nc.scalar.activation(
                        sqq[:, t * DQ : (t + 1) * DQ],
                        qr[:, t * DQ : (t + 1) * DQ],
                        AF.Square, accum_out=stq[:, 4 + t : 5 + t],
                    )
                nc.vector.tensor_scalar(
                    stq[:, 8:12], stq[:, 0:4], 1.0 / DQ, None, TT.mult
                )
                nc.vector.tensor_tensor(
                    stq[:, 12:16], stq[:, 8:12], stq[:, 0:4], TT.mult
                )
                nc.vector.tensor_tensor(
                    stq[:, 16:20], stq[:, 4:8], stq[:, 12:16], TT.subtract
                )
                nc.vector.tensor_scalar(
                    stq[:, 16:20], stq[:, 16:20], 1.0 / DQ, None, TT.mult
                )
                nc.scalar.activation(
                    stq[:, 20:24], stq[:, 16:20], AF.Sqrt, bias=eps_ap
                )
                nc.vector.reciprocal(stq[:, 20:24], stq[:, 20:24])
                nc.vector.scalar_tensor_tensor(
                    stq[:, 24:28], stq[:, 8:12], -1.0, stq[:, 20:24],
                    TT.mult, TT.mult,
                )
                qn4 = qw.tile([P, HG, DQ], F32, tag="qn4")
                for t in range(HG):
                    nc.scalar.activation(
                        qn4[:, t, :], qr[:, t * DQ : (t + 1) * DQ],
                        AF.Identity,
                        bias=stq[:, 24 + t : 25 + t],
                        scale=stq[:, 20 + t : 21 + t],
                    )
                cq = ropeq_sb[:, ic, 0 : HG * P].rearrange(
                    "p (h f) -> p h f", h=HG)
                sqt = ropeq_sb[:, ic, HG * P : 2 * HG * P].rearrange(
                    "p (h f) -> p h f", h=HG)
                r1 = qw.tile([P, HG, DQ], F32, tag="qr1")
                r2 = qw.tile([P, HG, DQ], F32, tag="qr2")
                nc.vector.tensor_tensor(r1[:], qn4[:], cq, TT.mult)
                nc.gpsimd.tensor_tensor(
                    r2[:, :, 0:HALF], qn4[:, :, HALF:DQ],
                    sqt[:, :, 0:HALF], TT.mult,
                )
                nc.gpsimd.tensor_tensor(
                    r2[:, :, HALF:DQ], qn4[:, :, 0:HALF],
                    sqt[:, :, HALF:DQ], TT.mult,
                )
                qrope = qw.tile([P, HG, DQ], F32, tag="qrope")
                nc.vector.tensor_tensor(qrope[:], r1[:], r2[:], TT.add)
                if has_qb:
                    bq4 = ropeq_sb[:, ic, 2 * HG * P : 3 * HG * P
                                   ].rearrange("p (h f) -> p h f", h=HG)
                    nc.vector.tensor_tensor(qrope[:], qrope[:], bq4,
                                            TT.add)
                for t in range(HG):
                    scr = kvtp.tile([P, P], F32, tag="scr")
                    nc.tensor.transpose(scr[:], qrope[:, t, :], ident)
                    nc.vector.tensor_copy(
                        qT_sb[:, g * HG + t, ic * P : (ic + 1) * P],
                        scr[:],
                    )

            q_iters = [(g, ic) for g in range(2) for ic in range(SC)]
            qi = 0

            kn4 = None
            xslab = None
            for jc in range(JC):
                if jc % 4 == 0:
                    kn4 = kn4p.tile([P, 4, P], F32, tag="kn4")
                if jc == 0:
                    xslab = xslab0
                elif jc % 2 == 0:
                    xslab = xsp.tile([P, DC, 2 * P], BF16, tag="xslab")
                    nc.sync.dma_start(
                        xslab[:],
                        x3[:, :, jc * P : (jc + 2) * P],
                    )
                if jc == 2:
                    nc.sync.dma_start(
                        xq_sb[:],
                        xq_in.rearrange("(c p) s -> p c s", p=P),
                    )
                elif jc == 4:
                    nc.sync.dma_start(
                        wq_sb[:],
                        wq.rearrange("(c p) f -> p c f", p=P),
                    )
                elif jc == 6:
                    nc.sync.dma_start(
                        ropeq_sb[:], ropeq.rearrange("(s p) f -> p s f", p=P)
                    )
                elif jc == 8:
                    nc.sync.dma_start(
                        biasT_sb[:], biasT.rearrange("(j p) i -> p j i", p=P)
                    )
                xc = xslab[:, :, (jc % 2) * P : (jc % 2 + 1) * P]
                kv_ps = kvps.tile([P, KV], F32, tag="kv_ps")
                for dc in range(DC):
                    nc.tensor.matmul(
                        kv_ps[:],
                        xc[:, dc, :],
                        wkv_sb[:, dc, :],
                        start=(dc == 0),
                        stop=(dc == DC - 1),
                    )
                # evict + folded rms1 bias; accum gives the LN sums free
                kvr = kvw.tile([P, KV], F32, tag="kvr")
                st = kvw.tile([P, 16], F32, tag="st")
                nc.vector.scalar_tensor_tensor(
                    kvr[:, :DK], kv_ps[:, :DK], 1.0,
                    cst[:, C_BKV : C_BKV + DK], TT.mult, TT.add,
                    accum_out=st[:, 0:1],
                )
                nc.vector.scalar_tensor_tensor(
                    kvr[:, DK:], kv_ps[:, DK:], 1.0,
                    cst[:, C_BKV + DK : C_BKV + KV], TT.mult, TT.add,
                    accum_out=st[:, 1:2],
                )
                # sumsq on ACT (square with accumulator)
                sq = kvw.tile([P, KV], F32, tag="sq")
                nc.scalar.activation(sq[:, :DK], kvr[:, :DK], AF.Square,
                                     accum_out=st[:, 2:3])
                nc.scalar.activation(sq[:, DK:], kvr[:, DK:], AF.Square,
                                     accum_out=st[:, 3:4])
                # smu = s1*invn ; var = (s2 - smu*s1)*invn ; rs = rsqrt(var+eps)
                nc.vector.tensor_tensor(st[:, 4:6], st[:, 0:2], invn, TT.mult)
                nc.vector.tensor_tensor(st[:, 6:8], st[:, 4:6], st[:, 0:2],
                                        TT.mult)
                nc.vector.tensor_tensor(st[:, 8:10], st[:, 2:4], st[:, 6:8],
                                        TT.subtract)
                nc.vector.tensor_tensor(st[:, 8:10], st[:, 8:10], invn,
                                        TT.mult)
                nc.scalar.activation(st[:, 10:12], st[:, 8:10], AF.Sqrt,
                                     bias=eps_ap)
                nc.vector.reciprocal(st[:, 10:12], st[:, 10:12])
                # nmr = -smu*rs  (bias for the ACT-side LN apply)
                nc.vector.scalar_tensor_tensor(
                    st[:, 12:14], st[:, 4:6], -1.0, st[:, 10:12],
                    TT.mult, TT.mult,
                )
                # apply (ACT): out = in*rs + (-smu*rs)
                nc.scalar.activation(
                    kn4[:, jc % 4, :], kvr[:, :DK], AF.Identity,
                    bias=st[:, 12:13], scale=st[:, 10:11],
                )
                nc.scalar.activation(
                    vrow_sb[:, jc, :DV], kvr[:, DK:], AF.Identity,
                    bias=st[:, 13:14], scale=st[:, 11:12],
                )
                if jc % 4 == 3:
                    j0 = jc - 3
                    ck = ropek_sb[:, j0 : j0 + 4, 0:P]
                    sk = ropek_sb[:, j0 : j0 + 4, P : 2 * P]
                    r1 = kvw.tile([P, 4, P], F32, tag="r1")
                    r2 = kvw.tile([P, 4, P], F32, tag="r2")
                    nc.vector.tensor_tensor(r1[:], kn4[:], ck, TT.mult)
                    nc.gpsimd.tensor_tensor(
                        r2[:, :, 0:HALF], kn4[:, :, HALF:P],
                        sk[:, :, 0:HALF], TT.mult,
                    )
                    nc.gpsimd.tensor_tensor(
                        r2[:, :, HALF:P], kn4[:, :, 0:HALF],
                        sk[:, :, HALF:P], TT.mult,
                    )
                    kr = kvw.tile([P, 4, P], F32, tag="kr")
                    nc.vector.tensor_tensor(kr[:], r1[:], r2[:], TT.add)
                    if has_kb:
                        bk = ropek_sb[:, j0 : j0 + 4, 2 * P : 3 * P]
                        nc.vector.tensor_tensor(kr[:], kr[:], bk, TT.add)
                    for t in range(4):
                        scr = kvtp.tile([P, P], F32, tag="scr")
                        nc.tensor.transpose(scr[:], kr[:, t, :], ident)
                        nc.vector.tensor_copy(kT_sb[:, j0 + t, :], scr[:])

                if jc >= 10 and qi < len(q_iters):
                    emit_q(*q_iters[qi])
                    qi += 1


            while qi < len(q_iters):
                emit_q(*q_iters[qi])
                qi += 1

        qres_cm.__exit__(None, None, None)

        # =====================================================
        # Attention
        # =====================================================
        with tc.tile_pool(name="wop", bufs=1) as wop:
            bor_sb = wop.tile([P, D], F32)
            nc.sync.dma_start(bor_sb[:], bout)
            yatt_sb = wop.tile([P, SC, H * DV], BF16)  # [q, sc, hdv]
            wo_sb = wop.tile([P, DC, D], BF16)
            nc.sync.dma_start(
                wo_sb[:], wo.rearrange("(c p) f -> p c f", p=P)
            )

            with (
                tc.tile_pool(name="att", bufs=2) as att,
                tc.tile_pool(name="apsum", bufs=2, space="PSUM") as aps,
                tc.tile_pool(name="ypsum", bufs=1, space="PSUM") as yps,
            ):
                y_pp = [
                    yps.tile([P, 2, VP], F32, tag=f"yp{j}", name=f"yp{j}")
                    for j in range(2)
                ]
                yT_all = wop.tile([P, SC, DC, P], BF16)
                UNLOCK = {0: [0], 1: [1, 2], 2: [3], 3: [4, 5],
                          4: [6], 5: [7, 8], 6: [9], 7: [10, 11]}
                for h in range(H):
                    for jp in range(JC // 4):      # pairs of 2-chunk groups
                        zt4 = att.tile([P, 4, SQ], BF16, tag="zt4")
                        for half in range(2):
                            jg = jp * 2 + half
                            pq = aps.tile([P, 2, SQ], F32, tag="pq")
                            for c in range(2):
                                jc = jg * 2 + c
                                nc.tensor.matmul(
                                    pq[:, c, :],
                                    kT_sb[:, jc, :],
                                    qT_sb[:, h, :],
                                    start=True, stop=True,
                                )
                            nc.vector.scalar_tensor_tensor(
                                zt4[:, half * 2 : half * 2 + 2, :],
                                pq[:],
                                1.0 / SOFTCAP,
                                biasT_sb[:, jg * 2 : jg * 2 + 2, :],
                                TT.mult, TT.add,
                            )
                        tt4 = att.tile([P, 4, SQ], BF16, tag="tt4")
                        nc.scalar.activation(tt4[:], zt4[:], AF.Tanh)
                        pt4 = att.tile([P, 4, SQ], BF16, tag="pt4")
                        nc.scalar.activation(pt4[:], tt4[:], AF.Exp,
                                             scale=SOFTCAP)
                        for c in range(4):
                            jc = jp * 4 + c
                            for ic in range(SC):
                                # bank shared per ic-pair: only the even ic
                                # start=True (clears the whole bank); the odd
                                # ic's first matmul overwrites via cleared
                                # has_written bits.
                                nc.tensor.matmul(
                                    y_pp[ic // 2][:, ic % 2, :],
                                    pt4[:, c, ic * P : (ic + 1) * P],
                                    vrow_sb[:, jc, :],
                                    start=(jc == 0 and ic % 2 == 0),
                                    stop=(jc == JC - 1),
                                    skip_group_check=True,
                                )
                    for ic in range(SC):
                        recip = att.tile([P, 1], F32, tag="recip")
                        nc.vector.reciprocal(
                            recip[:], y_pp[ic // 2][:, ic % 2, DV : DV + 1]
                        )
                        nc.vector.tensor_scalar(
                            yatt_sb[:, ic, h * DV : (h + 1) * DV],
                            y_pp[ic // 2][:, ic % 2, :DV],
                            recip[:, 0:1], None, TT.mult,
                        )
                    # out-proj transposes for feature chunks whose heads are
                    # all drained now (overlaps the ACT-bound attention)
                    for fc in UNLOCK[h]:
                        for sc in range(SC):
                            tp = aps.tile([P, P], BF16, tag="ytp")
                            nc.tensor.transpose(
                                tp[:],
                                yatt_sb[:, sc, fc * P : (fc + 1) * P],
                                ident_bf[:],
                            )
                            nc.vector.tensor_copy(
                                yT_all[:, sc, fc, :], tp[:]
                            )

            # =================================================
            # Output projection
            # =================================================
            with (
                tc.tile_pool(name="op", bufs=2) as op,
                tc.tile_pool(name="opsum", bufs=2, space="PSUM") as ops,
            ):
                for sc in range(SC):
                    yT = yT_all[:, sc, :, :]
                    o_ps = [
                        ops.tile([P, 512], F32, tag=f"o{n}", name=f"o{n}_{sc}")
                        for n in range(3)
                    ]
                    for fc in range(DC):
                        for n in range(3):
                            nc.tensor.matmul(
                                o_ps[n][:],
                                yT[:, fc, :],
                                wo_sb[:, fc, n * 512 : (n + 1) * 512],
                                start=(fc == 0),
                                stop=(fc == DC - 1),
                            )
                    o_sb = op.tile([P, D], F32, tag="o_sb")
                    for n in range(3):
                        nc.vector.tensor_tensor(
                            o_sb[:, n * 512 : (n + 1) * 512],
                            o_ps[n][:],
                            bor_sb[:, n * 512 : (n + 1) * 512],
                            TT.add,
                        )
                    nc.sync.dma_start(out[sc * P : (sc + 1) * P, :], o_sb[:])

    nc.compile()
    return nc


def _rope_tables(n, g, b, scale, start=0):
    """Full-width tables (f32): out = xhat*C + xswap*Sw (+ B)."""
    f32 = np.float32
    freqs = 1.0 / (ROPE_BASE ** (np.arange(HALF, dtype=f32) / HALF))
    ang = (start + np.arange(n, dtype=f32))[:, None] * freqs[None, :]
    cos, sin = np.cos(ang).astype(f32), np.sin(ang).astype(f32)
    g1, g2 = g[:HALF], g[HALF:]
    b1, b2 = b[:HALF], b[HALF:]
    C = np.concatenate([g1 * cos, g2 * cos], axis=1) * scale
    Sw = np.concatenate([-g2 * sin, g1 * sin], axis=1) * scale
    Bt = np.concatenate([b1 * cos - b2 * sin, b1 * sin + b2 * cos],
                        axis=1) * scale
    return C.astype(f32), Sw.astype(f32), Bt.astype(f32)


def _host_prep(inputs):
    f32 = np.float32
    x = np.asarray(inputs["x"], f32)
    bias = np.asarray(inputs["attention_bias"], f32)
    g1 = np.asarray(inputs["g1"], f32)
    b1 = np.asarray(inputs["b1"], f32)
    rr1 = np.asarray(inputs["rrms1"], f32)
    Wq = np.asarray(inputs["Wq"], f32)
    Wk = np.asarray(inputs["Wk"], f32)
    Wv = np.asarray(inputs["Wv"], f32)
    qg = np.asarray(inputs["qg"], f32)
    qb = np.asarray(inputs["qb"], f32)
    kg = np.asarray(inputs["kg"], f32)
    kb = np.asarray(inputs["kb"], f32)
    vg = np.asarray(inputs["vg"], f32)
    vb = np.asarray(inputs["vb"], f32)
    Wo = np.asarray(inputs["Wo"], f32)
    bo = np.asarray(inputs["bo"], f32)
    g2 = np.asarray(inputs["g2"], f32)
    b2 = np.asarray(inputs["b2"], f32)
    rr2 = np.asarray(inputs["rrms2"], f32)

    has_kb = bool(np.any(kb != 0))
    has_qb = bool(np.any(qb != 0))

    scale1 = (g1 * (1.0 / np.sqrt(rr1 + EPS_RMS))).astype(f32)
    Wkv = np.concatenate([Wk * scale1[:, None], Wv * scale1[:, None]],
                         axis=1).astype(f32)
    bkv = np.concatenate([b1 @ Wk, b1 @ Wv]).astype(f32)
    Wq_e = (Wq * scale1[:, None]).astype(f32)
    bq_row = (b1 @ Wq).astype(f32)

    sc_q = f32(DQ) ** f32(-0.5)
    scale2 = (g2 * (1.0 / np.sqrt(rr2 + EPS_RMS))).astype(f32)
    vg_t = np.tile(vg, H).astype(f32)
    vb_t = np.tile(vb, H).astype(f32)
    Wo_f = (vg_t[:, None] * Wo * scale2[None, :]).astype(f32)
    bo_f = ((vb_t @ Wo + bo) * scale2 + b2).astype(f32)

    # k rope tables (full S)
    Ck, Sk, Bk = _rope_tables(S, kg, kb, f32(1.0))
    ropek_cols = [Ck, Sk] + ([Bk] if has_kb else [])
    ropek = np.concatenate(ropek_cols, axis=1).astype(NPBF16)

    rep = lambda v: np.ascontiguousarray(
        np.broadcast_to(v[None, :], (P, v.shape[0])), dtype=f32)
    consts = np.zeros((P, CW), f32)
    consts[:, C_ID : C_ID + P] = np.eye(P, dtype=f32)
    consts[:, C_BKV : C_BKV + KV] = rep(bkv)
    consts[:, C_BQ : C_BQ + H * DQ] = rep(bq_row)
    consts[:, C_INV] = f32(1.0 / DK)
    consts[:, C_INV + 1] = f32(1.0 / DV)
    consts[:, C_EPS] = f32(EPS_LN)

    shared = {
        "wkv": np.ascontiguousarray(Wkv.astype(NPBF16)),
        "wq": np.ascontiguousarray(Wq_e.astype(NPBF16)),
        "wo": np.ascontiguousarray(Wo_f.astype(NPBF16)),
        "ropek": np.ascontiguousarray(ropek),
        "consts": np.ascontiguousarray(consts),
        "bout": rep(bo_f),
    }

    xTs = [np.ascontiguousarray(x[b].T.astype(NPBF16)) for b in range(B)]
    in_maps = []
    for c in range(NCORES):
        b = c // 4
        s0 = (c % 4) * SQ
        m = dict(shared)
        m["xT"] = xTs[b]
        m["xq"] = np.ascontiguousarray(xTs[b][:, s0 : s0 + SQ])
        m["biasT"] = np.ascontiguousarray(
            (bias[0, 0, s0 : s0 + SQ, :].T * (1.0 / SOFTCAP)).astype(NPBF16)
        )
        Cq, Sq, Bq = _rope_tables(SQ, qg, qb, sc_q, start=s0)
        rq_cols = [np.tile(Cq, (1, HG)), np.tile(Sq, (1, HG))]
        if has_qb:
            rq_cols.append(np.tile(Bq, (1, HG)))
        m["ropeq"] = np.ascontiguousarray(
            np.concatenate(rq_cols, axis=1).astype(NPBF16)
        )
        in_maps.append(m)
    return in_maps, (has_kb, has_qb)


_NC_CACHE = {}


def _get_nc(flags=(False, False)):
    if flags not in _NC_CACHE:
        _NC_CACHE[flags] = build_program(*flags)
    return _NC_CACHE[flags]


def kernel(**inputs) -> np.ndarray:
    in_maps, flags = _host_prep(inputs)
    nc = _get_nc(flags)
    res = bass_utils.run_bass_kernel_spmd(
        nc, in_maps, core_ids=list(range(NCORES))
    )
    outs = res.results
    full = np.empty((B, S, D), np.float32)
    for c in range(NCORES):
        b = c // 4
        s0 = (c % 4) * SQ
        full[b, s0 : s0 + SQ, :] = outs[c]["out"]
    return full


if __name__ == "__main__":
    nc = _get_nc()
    print("build + compile OK")
